# revision 1
# baseline (speedup 1.0000x reference)
"""DiscreteMamba2 fused kernel for 8 trn2 NeuronCores.

Sharding: data parallel on batch (2) x tensor parallel on heads (4 groups of 8
heads). Core c handles batch b=c//4, head group g=c%4. Per core: in_proj slice
(x|B|C|z|A_log channels for its heads), depthwise causal conv (as 4 diagonal
matmuls), chunked SSD (Q=256), gating, then AllGather of y^T within the batch
group and a 512-column slice of out_proj. Host assembles the full output.
"""
import sys

sys.path.insert(0, "/opt/trn_rl_repo")

import numpy as np
from contextlib import ExitStack

import concourse.bass as bass
import concourse.mybir as mybir
import concourse.tile as tile
from concourse import bacc
from concourse.bass_utils import run_bass_kernel_spmd

F32 = mybir.dt.float32
F32R = mybir.dt.float32r
I32 = mybir.dt.int32
AF = mybir.ActivationFunctionType
OP = mybir.AluOpType

D_MODEL = 2048
D_INNER = 2048
D_STATE = 64
NH = 32
HEADDIM = 64
D_CONV = 4
CONV_DIM = D_INNER + 2 * NH * D_STATE          # 6144
IN_DIM = 2 * D_INNER + 2 * NH * D_STATE + NH   # 8224
B_SZ, L = 2, 1024

NHC = 8              # heads per core
CH = NHC * 64        # 512 channels per block (x, B, C, z each)
NCH_T = 17           # in_proj channel tiles per core (16 full + A_log tile)
NK = 16              # d_model k tiles
Q = 256              # SSD chunk length
NCHUNK = L // Q      # 4
NBLK = L // 128      # 8 token blocks
NEG = -1.0e30


def _build_program(debug=False):
    nc = bacc.Bacc("TRN2", target_bir_lowering=False, debug=False, num_devices=8)

    # ---- dram I/O ----
    uT_d = nc.dram_tensor("uT", [NK, 128, L], F32, kind="ExternalInput")
    w_d = nc.dram_tensor("w", [NCH_T, 128, NK, 128], F32, kind="ExternalInput")
    cw_d = nc.dram_tensor("cw", [12, 128, D_CONV, 128], F32, kind="ExternalInput")
    cb_d = nc.dram_tensor("cb", [12, 128], F32, kind="ExternalInput")
    zb_d = nc.dram_tensor("zb", [4, 128], F32, kind="ExternalInput")
    dd_d = nc.dram_tensor("dd", [1, NHC], F32, kind="ExternalInput")
    wo_d = nc.dram_tensor("wo", [NK, 128, CH], F32, kind="ExternalInput")
    mask_d = nc.dram_tensor("maskn", [128, 2, Q], F32, kind="ExternalInput")
    ident_d = nc.dram_tensor("ident", [128, 128], F32, kind="ExternalInput")
    sel_d = nc.dram_tensor("sel", [8, NHC, 128], F32, kind="ExternalInput")
    out_d = nc.dram_tensor("out", [L, CH], F32, kind="ExternalOutput")
    if debug:
        dbg_xbc = nc.dram_tensor("dbg_xbc", [128, 12, L], F32, kind="ExternalOutput")
        dbg_zt = nc.dram_tensor("dbg_zt", [128, 4, L], F32, kind="ExternalOutput")
        dbg_arow = nc.dram_tensor("dbg_arow", [NHC, L], F32, kind="ExternalOutput")
        dbg_cum = nc.dram_tensor("dbg_cum", [NHC, L], F32, kind="ExternalOutput")
        dbg_xf = nc.dram_tensor("dbg_xf", [128, NBLK, CH], F32, kind="ExternalOutput")
        dbg_bdb = nc.dram_tensor("dbg_bdb", [128, NBLK, CH], F32, kind="ExternalOutput")
        dbg_ygt = nc.dram_tensor("dbg_ygt", [128, 4, L], F32, kind="ExternalOutput")
        dbg_ago = nc.dram_tensor("dbg_ago", [D_INNER, L], F32, kind="ExternalOutput")
        dbg_mt = nc.dram_tensor("dbg_mt", [128, 2, Q], F32, kind="ExternalOutput")
        dbg_sa = nc.dram_tensor("dbg_sa", [128, 2, Q], F32, kind="ExternalOutput")
        dbg_cumat = nc.dram_tensor("dbg_cumat", [128, NBLK, NHC], F32, kind="ExternalOutput")
        dbg_rb = nc.dram_tensor("dbg_rb", [128, L], F32, kind="ExternalOutput")
        dbg_sg = nc.dram_tensor("dbg_sg", [NHC, L], F32, kind="ExternalOutput")
        dbg_av = nc.dram_tensor("dbg_av", [128, NHC * L // 128], F32, kind="ExternalOutput")
        dbg_at = nc.dram_tensor("dbg_at", [NHC, L], F32, kind="ExternalOutput")

    # internal scratch (AllGather split into two channel-halves for overlap)
    ag_in1 = nc.dram_tensor("ag_in1", [CH // 2, L], F32)
    ag_in2 = nc.dram_tensor("ag_in2", [CH // 2, L], F32)
    ag_out1 = nc.dram_tensor("ag_out1", [4 * CH // 2, L], F32)
    ag_out2 = nc.dram_tensor("ag_out2", [4 * CH // 2, L], F32)

    with tile.TileContext(nc) as tc, ExitStack() as ctx:
        persist = ctx.enter_context(tc.tile_pool(name="persist", bufs=1))

        # persistent tiles
        xbc = persist.tile([128, 12, L], F32R, tag="xbc")     # conv output: x|B|C, ch on partitions
        zt = persist.tile([128, 4, L], F32, tag="zt")          # z^T
        maskn = persist.tile([128, 2, Q], F32, tag="maskn")
        ident = persist.tile([128, 128], F32R, tag="ident")
        cumat = persist.tile([128, NBLK, NHC], F32, tag="cumat")
        decb = persist.tile([128, NHC, NCHUNK], F32, tag="decb")
        ddb = persist.tile([128, NHC], F32, tag="ddb")
        cb_t = persist.tile([128, 12], F32, tag="cb")
        zb_t = persist.tile([128, 4], F32, tag="zb")
        arow = persist.tile([NHC, L], F32, tag="arow")
        cum = persist.tile([NHC, L], F32, tag="cum")
        zer = persist.tile([NHC, Q], F32, tag="zer")

        nc.sync.dma_start(out=maskn, in_=mask_d.ap())
        nc.sync.dma_start(out=ident, in_=ident_d.ap().bitcast(F32R))
        sel = persist.tile([8, NHC, 128], F32, tag="sel")
        nc.sync.dma_start(out=sel, in_=sel_d.ap())
        nc.sync.dma_start(out=cb_t, in_=cb_d.ap().transpose([1, 0]))
        nc.sync.dma_start(out=zb_t, in_=zb_d.ap().transpose([1, 0]))
        nc.vector.memset(zer, 0.0)

        small_ctx = ExitStack()
        small = small_ctx.enter_context(tc.tile_pool(name="small", bufs=1))

        # ---------------- P1: in_proj + fused conv ----------------
        with tc.tile_pool(name="uT_pool", bufs=1) as uT_pool, \
             tc.tile_pool(name="w_pool", bufs=2) as w_pool, \
             tc.tile_pool(name="pre_pool", bufs=2) as pre_pool, \
             tc.tile_pool(name="psum", bufs=3, space="PSUM") as psum:
            uT = uT_pool.tile([128, NK, L], F32R, tag="uT")
            for k in range(NK):
                nc.sync.dma_start(out=uT[:, k, :], in_=uT_d.ap().bitcast(F32R)[k])

            for ct in [NCH_T - 1] + list(range(NCH_T - 1)):
                wsl = w_pool.tile([128, NK, 128], F32R, tag="w")
                nc.sync.dma_start(out=wsl, in_=w_d.ap().bitcast(F32R)[ct])
                if ct < 12:
                    pre = pre_pool.tile([128, L + D_CONV - 1], F32R, tag="pre")
                    nc.vector.memset(pre.bitcast(F32)[:, 0:D_CONV - 1], 0.0)
                    cwt = w_pool.tile([128, D_CONV, 128], F32R, tag="cw")
                    nc.sync.dma_start(out=cwt, in_=cw_d.ap().bitcast(F32R)[ct])
                for tch in range(2):
                    pp = psum.tile([128, 512], F32, tag="proj")
                    for k in range(NK):
                        nc.tensor.matmul(pp, wsl[:, k, :], uT[:, k, tch * 512:(tch + 1) * 512],
                                         start=(k == 0), stop=(k == NK - 1))
                    if ct < 12:
                        nc.scalar.copy(out=pre[:, D_CONV - 1 + tch * 512:D_CONV - 1 + (tch + 1) * 512], in_=pp)
                    elif ct < 16:
                        nc.scalar.copy(out=zt[:, ct - 12, tch * 512:(tch + 1) * 512], in_=pp)
                    else:
                        nc.vector.tensor_copy(arow[:, tch * 512:(tch + 1) * 512], pp[0:NHC, :])
                if ct < 12:
                    for tch in range(2):
                        cp = psum.tile([128, 512], F32, tag="conv")
                        for k in range(D_CONV):
                            nc.tensor.matmul(cp, cwt[:, k, :], pre[:, tch * 512 + k: tch * 512 + k + 512],
                                             start=(k == 0), stop=(k == D_CONV - 1))
                        nc.scalar.activation(xbc[:, ct, tch * 512:(tch + 1) * 512], cp,
                                             AF.Identity, bias=cb_t[:, ct:ct + 1], scale=1.0)

        # ---------------- P2: a = ln(sigmoid(-A_log)), chunked cumsum, decay vectors ----------------
        sg = small.tile([NHC, L], F32, tag="sg")
        nc.scalar.activation(sg, arow, AF.Sigmoid, bias=0.0, scale=-1.0)
        # ln(sg) via exponent/mantissa split + atanh series, on [NHC, L]
        ei = small.tile([NHC, L], I32, tag="ln1")
        nc.vector.tensor_scalar(ei, sg.bitcast(I32), 23, None, OP.logical_shift_right)
        ef = small.tile([NHC, L], F32, tag="ef")
        nc.vector.tensor_copy(ef, ei)
        nc.vector.tensor_scalar(ef, ef, 127.0, None, OP.subtract)
        mi = small.tile([NHC, L], I32, tag="ln2")
        nc.vector.tensor_scalar(mi, sg.bitcast(I32), 0x007FFFFF, 0x3F800000,
                                OP.bitwise_and, OP.bitwise_or)
        m_ = mi.bitcast(F32)
        t1 = small.tile([NHC, L], F32, tag="ln1")
        nc.vector.tensor_scalar(t1, m_, 1.0, None, OP.subtract)
        t2 = small.tile([NHC, L], F32, tag="ln3")
        nc.vector.tensor_scalar(t2, m_, 1.0, None, OP.add)
        nc.vector.reciprocal(t2, t2)
        z_ = small.tile([NHC, L], F32, tag="ln2")
        nc.vector.tensor_tensor(z_, t1, t2, OP.mult)
        z2 = small.tile([NHC, L], F32, tag="ln1")
        nc.vector.tensor_tensor(z2, z_, z_, OP.mult)
        w_ = small.tile([NHC, L], F32, tag="ln3")
        nc.vector.tensor_scalar(w_, z2, 1.0 / 9.0, None, OP.mult)
        for cc in (1.0 / 7.0, 1.0 / 5.0, 1.0 / 3.0):
            nc.vector.scalar_tensor_tensor(w_, w_, cc, z2, OP.add, OP.mult)
        nc.vector.scalar_tensor_tensor(w_, w_, 1.0, z_, OP.add, OP.mult)
        nc.vector.tensor_scalar(w_, w_, 2.0, None, OP.mult)
        at = small.tile([NHC, L], F32, tag="ln2")
        nc.vector.scalar_tensor_tensor(at, ef, float(np.log(2.0)), w_, OP.mult, OP.add)

        # chunk-local inclusive cumsums
        for c in range(NCHUNK):
            nc.vector.tensor_tensor_scan(cum[:, c * Q:(c + 1) * Q], at[:, c * Q:(c + 1) * Q],
                                         zer, 0.0, OP.add, OP.add)

        # din = exp(cum); db = exp(cum_end - cum); dec = exp(cum_end)
        din = persist.tile([NHC, L], F32, tag="din")
        nc.scalar.activation(din, cum, AF.Exp, bias=0.0, scale=1.0)
        dbt = small.tile([NHC, L], F32, tag="dbt")
        for c in range(NCHUNK):
            nc.vector.tensor_scalar(dbt[:, c * Q:(c + 1) * Q], cum[:, c * Q:(c + 1) * Q],
                                    cum[:, (c + 1) * Q - 1:(c + 1) * Q], None, OP.subtract)
        nc.scalar.activation(dbt, dbt, AF.Exp, bias=0.0, scale=-1.0)
        decr = small.tile([NHC, NCHUNK], F32, tag="decr")
        cum_r = cum.rearrange("h (c q) -> h c q", q=Q)
        nc.scalar.activation(decr, cum_r[:, :, Q - 1], AF.Exp, bias=0.0, scale=1.0)

        # transposed reads via PE transpose (on-chip, race-free)
        with tc.tile_pool(name="psum_p2", bufs=2, space="PSUM") as psum_p2:
            for blk in range(NBLK):
                tpc = psum_p2.tile([128, NHC], F32, tag="tpc")
                nc.tensor.transpose(tpc, cum[:, blk * 128:(blk + 1) * 128],
                                    ident.bitcast(F32)[0:NHC, 0:NHC])
                nc.vector.tensor_copy(cumat[:, blk, :], tpc)
            dbT = persist.tile([128, NBLK, NHC], F32, tag="dbT")
            for blk in range(NBLK):
                tpd = psum_p2.tile([128, NHC], F32, tag="tpd")
                nc.tensor.transpose(tpd, dbt[:, blk * 128:(blk + 1) * 128],
                                    ident.bitcast(F32)[0:NHC, 0:NHC])
                nc.vector.tensor_copy(dbT[:, blk, :], tpd)
            for h in range(NHC):
                dps = psum_p2.tile([128, NCHUNK], F32, tag="tpc")
                nc.tensor.matmul(dps, sel[:, h, :], decr, start=True, stop=True)
                nc.vector.tensor_copy(decb[:, h, :], dps)
        nc.sync.dma_start(out=ddb, in_=bass.AP(tensor=dd_d.ap().tensor, offset=0,
                                               ap=[[0, 128], [1, NHC]]))

        # early silu(z + z_bias) = (z+zb)*sigmoid(z+zb), in place
        for ct in range(4):
            zs_ = small.tile([128, L], F32, tag="zs")
            nc.scalar.activation(zs_, zt[:, ct, :], AF.Sigmoid,
                                 bias=zb_t[:, ct:ct + 1], scale=1.0)
            nc.vector.tensor_scalar(zt[:, ct, :], zt[:, ct, :],
                                    zb_t[:, ct:ct + 1], None, OP.add)
            nc.vector.tensor_tensor(zt[:, ct, :], zt[:, ct, :], zs_, OP.mult)

        small_ctx.close()

        # ---------------- P3: transpose x and B blocks ----------------
        mid_ctx = ExitStack()
        mid = mid_ctx.enter_context(tc.tile_pool(name="mid", bufs=1))
        Xf = mid.tile([128, NBLK, CH], F32R, tag="Xf")     # x transposed [tok, ch]
        Bdb = mid.tile([128, NBLK, CH], F32R, tag="Bdb")   # B transposed * db [tok, ch]
        ygt = mid.tile([128, 4, L], F32, tag="ygt")        # gated y^T
        with tc.tile_pool(name="psum_tr", bufs=4, space="PSUM") as psum_tr:
          for src_ct, dst in ((0, Xf), (4, Bdb)):
            for ct in range(4):
                for blk in range(NBLK):
                    tp = psum_tr.tile([128, 128], F32R, tag="tr")
                    nc.tensor.transpose(tp, xbc[:, src_ct + ct, blk * 128:(blk + 1) * 128], ident)
                    if src_ct == 0:
                        nc.vector.tensor_copy(dst[:, blk, ct * 128:(ct + 1) * 128], tp)
                    else:
                        for hh in range(2):
                            h = ct * 2 + hh
                            nc.vector.tensor_scalar(
                                dst[:, blk, h * 64:(h + 1) * 64],
                                tp[:, hh * 64:(hh + 1) * 64],
                                dbT[:, blk, h:h + 1], None, OP.mult)

        # ---------------- P4: chunked SSD per head ----------------
        with tc.tile_pool(name="rb_pool", bufs=2) as rb_pool, \
             tc.tile_pool(name="ssd_pool", bufs=4) as ssd_pool, \
             tc.tile_pool(name="h_pool", bufs=3) as h_pool, \
             tc.tile_pool(name="psum_ssd", bufs=2, space="PSUM") as psum_ssd:
            for h in range(NHC):
                bct, po = 4 + h // 2, (h % 2) * 64
                cct = 8 + h // 2
                rb = rb_pool.tile([128, L], F32, tag="rb")
                dinb = rb_pool.tile([128, L], F32, tag="dinb")
                for half in range(2):
                    hsl = slice(half * 512, (half + 1) * 512)
                    bps = psum_ssd.tile([128, 512], F32, tag="bps", bufs=1)
                    nc.tensor.matmul(bps, sel[:, h, :], cum[:, hsl], start=True, stop=True)
                    nc.vector.tensor_copy(rb[:, hsl], bps)
                    bps2 = psum_ssd.tile([128, 512], F32, tag="bps", bufs=1)
                    nc.tensor.matmul(bps2, sel[:, h, :], din[:, hsl], start=True, stop=True)
                    nc.vector.tensor_copy(dinb[:, hsl], bps2)
                if debug and h == 0:
                    import os as _os4
                    if "rb" in _os4.environ.get("KDBG", "").split(","):
                        nc.sync.dma_start(out=dbg_rb.ap(), in_=rb)
                        nc.sync.dma_start(out=dbg_cumat.ap(), in_=cumat)
                hcur = h_pool.tile([128, 64], F32R, tag="ha")
                hnxt = h_pool.tile([128, 64], F32R, tag="hb")
                nc.vector.memset(hcur.bitcast(F32)[po:po + 64, :], 0.0)
                for c in range(NCHUNK):
                    csl = slice(c * Q, (c + 1) * Q)
                    mts = []
                    for st in range(2):
                        blk = 2 * c + st
                        cbp_f = psum_ssd.tile([128, 512], F32, tag="cb")
                        cbp = cbp_f[:, 0:Q]
                        nc.tensor.matmul(cbp,
                                         xbc[po:po + 64, bct, blk * 128:(blk + 1) * 128],
                                         xbc[po:po + 64, cct, csl],
                                         start=True, stop=True)
                        sA = ssd_pool.tile([128, Q], F32, tag="sA")
                        lo = 128 * st
                        if st == 1:
                            nc.vector.memset(sA[:, 0:128], 0.0)
                        nc.vector.scalar_tensor_tensor(
                            sA[:, lo:], rb[:, c * Q + lo:(c + 1) * Q],
                            cumat[:, blk, h:h + 1], maskn[:, st, lo:],
                            OP.subtract, OP.add)
                        nc.scalar.activation(sA[:, lo:], sA[:, lo:], AF.Exp, bias=0.0, scale=1.0)
                        mt = ssd_pool.tile([128, Q], F32R, tag="mt")
                        nc.vector.tensor_tensor(mt, cbp, sA, OP.mult)
                        mts.append(mt)
                        if debug and h == 0 and c == 0:
                            import os as _os3
                            if "mt" in _os3.environ.get("KDBG", "").split(","):
                                nc.sync.dma_start(out=dbg_mt.ap()[:, st, :], in_=mt.bitcast(F32))
                                nc.sync.dma_start(out=dbg_sa.ap()[:, st, :], in_=sA)
                    # Y^T intra [hd, l] (psum base 0)
                    yt_f = psum_ssd.tile([128, 512], F32, tag="yt")
                    yt = yt_f[0:64, 0:Q]
                    for st in range(2):
                        nc.tensor.matmul(yt, Xf[:, 2 * c + st, h * 64:(h + 1) * 64], mts[st],
                                         start=(st == 0), stop=(st == 1))
                    # Y^T inter raw [hd, l]: lhsT = H slice at po, rhs = C^T at po
                    yi_f = psum_ssd.tile([128, 512], F32, tag="yi", bufs=1)
                    yi = yi_f[0:64, 0:Q]
                    nc.tensor.matmul(yi, hcur[po:po + 64, :], xbc[po:po + 64, cct, csl],
                                     start=True, stop=True)
                    # state S_c and H update
                    sp_f = psum_ssd.tile([128, 512], F32, tag="sp")
                    sp = sp_f[0:64, 0:64]
                    for st in range(2):
                        nc.tensor.matmul(sp, Bdb[:, 2 * c + st, h * 64:(h + 1) * 64],
                                         Xf[:, 2 * c + st, h * 64:(h + 1) * 64],
                                         start=(st == 0), stop=(st == 1))
                    nc.vector.scalar_tensor_tensor(hnxt[po:po + 64, :], hcur.bitcast(F32)[po:po + 64, :],
                                                   decb[po:po + 64, h, c:c + 1], sp, OP.mult, OP.add)
                    hcur, hnxt = hnxt, hcur
                    if c < NCHUNK - 1:
                        hnxt = h_pool.tile([128, 64], F32R, tag=("ha" if c % 2 == 1 else "hb"))
                    # assemble: ygt_chunk = ((yi * din) + D*x_raw) + yt   (all at base po)
                    o1 = ssd_pool.tile([128, Q], F32, tag="o1")
                    nc.vector.tensor_tensor(o1[po:po + 64, :], dinb[po:po + 64, csl], yi, OP.mult)
                    o2 = ssd_pool.tile([128, Q], F32, tag="o2")
                    nc.vector.scalar_tensor_tensor(o2[po:po + 64, :],
                                                   xbc.bitcast(F32)[po:po + 64, h // 2, csl],
                                                   ddb[po:po + 64, h:h + 1], o1[po:po + 64, :],
                                                   OP.mult, OP.add)
                    nc.vector.tensor_tensor(ygt[po:po + 64, h // 2, csl], o2[po:po + 64, :],
                                            yt, OP.add)

        if debug:
            import os as _os
            _dumps = _os.environ.get("KDBG", "xbc,zt,arow,cum,xf,bdb,ygt").split(",")
            if "xbc" in _dumps:
                nc.sync.dma_start(out=dbg_xbc.ap(), in_=xbc.bitcast(F32))
            if "zt" in _dumps:
                nc.sync.dma_start(out=dbg_zt.ap(), in_=zt)
            if "arow" in _dumps:
                nc.sync.dma_start(out=dbg_arow.ap(), in_=arow)
            if "cum" in _dumps:
                nc.sync.dma_start(out=dbg_cum.ap(), in_=cum)
            if "xf" in _dumps:
                nc.sync.dma_start(out=dbg_xf.ap(), in_=Xf.bitcast(F32))
            if "bdb" in _dumps:
                nc.sync.dma_start(out=dbg_bdb.ap(), in_=Bdb)
            if "ygt" in _dumps:
                nc.sync.dma_start(out=dbg_ygt.ap(), in_=ygt)

        # ---------------- P5: gating + write to ag halves ----------------
        for ct in range(4):
            nc.vector.tensor_tensor(ygt[:, ct, :], ygt[:, ct, :], zt[:, ct, :], OP.mult)
            tgt = ag_in1 if ct < 2 else ag_in2
            nc.sync.dma_start(out=tgt.ap()[(ct % 2) * 128:(ct % 2) * 128 + 128, :],
                              in_=ygt[:, ct, :])
            if ct == 1:
                nc.gpsimd.collective_compute(
                    "AllGather", OP.bypass,
                    ins=[ag_in1.ap()], outs=[ag_out1.ap()],
                    replica_groups=[[0, 1, 2, 3], [4, 5, 6, 7]],
                )
        nc.gpsimd.collective_compute(
            "AllGather", OP.bypass,
            ins=[ag_in2.ap()], outs=[ag_out2.ap()],
            replica_groups=[[0, 1, 2, 3], [4, 5, 6, 7]],
        )

        mid_ctx.close()

        if debug:
            import os as _os2
            if "ago" in _os2.environ.get("KDBG", "").split(","):
                nc.sync.dma_start(out=dbg_ago.ap()[0:1024, :], in_=ag_out1.ap())
                nc.sync.dma_start(out=dbg_ago.ap()[1024:2048, :], in_=ag_out2.ap())

        # ---------------- P7: out_proj slice ----------------
        with tc.tile_pool(name="yT_pool", bufs=1) as yT_pool, \
             tc.tile_pool(name="wo_pool", bufs=2) as wo_pool, \
             tc.tile_pool(name="o_pool", bufs=3) as o_pool, \
             tc.tile_pool(name="psum_o", bufs=2, space="PSUM") as psum_o:
            yT = yT_pool.tile([128, NK, L], F32R, tag="yT")
            for k in range(NK):
                ao = ag_out1 if k < NK // 2 else ag_out2
                kk = k % (NK // 2)
                nc.sync.dma_start(out=yT[:, k, :],
                                  in_=ao.ap().bitcast(F32R)[kk * 128:(kk + 1) * 128, :])
            wos = []
            for k in range(NK):
                wsl = wo_pool.tile([128, CH], F32R, tag=f"wo{k}", bufs=1)
                nc.sync.dma_start(out=wsl, in_=wo_d.ap().bitcast(F32R)[k])
                wos.append(wsl)
            for tt in range(NBLK):
                op_ = psum_o.tile([128, 512], F32, tag="op")
                for k in range(NK):
                    nc.tensor.matmul(op_, yT[:, k, tt * 128:(tt + 1) * 128], wos[k],
                                     start=(k == 0), stop=(k == NK - 1))
                ob = o_pool.tile([128, CH], F32, tag="ob")
                nc.vector.tensor_copy(ob, op_)
                nc.sync.dma_start(out=out_d.ap()[tt * 128:(tt + 1) * 128, :], in_=ob)

    nc.compile()
    return nc


_NC_CACHE = None


def _get_program():
    global _NC_CACHE
    if _NC_CACHE is None:
        _NC_CACHE = _build_program()
    return _NC_CACHE


def _pack_core_inputs(c, u, W_in, conv_w, conv_b, z_bias, D, W_out, masks, ident, sel):
    b, g = c // 4, c % 4
    hs = slice(g * 512, (g + 1) * 512)
    # uT tiles [NK, 128, L]
    uT = np.ascontiguousarray(u[b].T.reshape(NK, 128, L))
    # W slice rows: x | B | C | z | A_log(+pad)
    rows = np.concatenate([
        W_in[hs], W_in[2048 + g * 512:2048 + (g + 1) * 512],
        W_in[4096 + g * 512:4096 + (g + 1) * 512],
        W_in[6144 + g * 512:6144 + (g + 1) * 512],
        W_in[8192 + g * NHC:8192 + (g + 1) * NHC],
        np.zeros((128 - NHC, D_MODEL), np.float32),
    ])  # (2176, 2048)
    # -> [NCH_T, NK, 128k, 128ch]
    w = np.ascontiguousarray(
        rows.T.reshape(NK, 128, NCH_T, 128).transpose(2, 1, 0, 3))
    # conv diag tiles [12, D_CONV, 128, 128] and bias [12, 128]
    cw_rows = np.concatenate([conv_w[hs, 0, :], conv_w[2048 + g * 512:2048 + (g + 1) * 512, 0, :],
                              conv_w[4096 + g * 512:4096 + (g + 1) * 512, 0, :]])  # (1536, 4)
    cw = np.zeros((12, 128, D_CONV, 128), np.float32)
    idx = np.arange(128)
    for ct in range(12):
        for k in range(D_CONV):
            cw[ct, idx, k, idx] = cw_rows[ct * 128:(ct + 1) * 128, k]
    cb = np.concatenate([conv_b[hs], conv_b[2048 + g * 512:2048 + (g + 1) * 512],
                         conv_b[4096 + g * 512:4096 + (g + 1) * 512]]).reshape(12, 128)
    zb = z_bias[hs].reshape(4, 128)
    dd = D[g * NHC:(g + 1) * NHC].reshape(1, NHC)
    # W_out slice: columns of out for this group; rows = d_inner in natural order
    wot = W_out[hs, :].T  # (2048 d_inner rows, 512 oc)
    perm = np.empty(2048, np.int64)
    for i in range(1024):
        r, j = i // 256, i % 256
        perm[i] = r * 512 + j
        perm[1024 + i] = r * 512 + 256 + j
    wo = np.ascontiguousarray(wot[perm].reshape(NK, 128, CH))
    return {
        "uT": uT.astype(np.float32), "w": w.astype(np.float32),
        "cw": cw, "cb": cb.astype(np.float32), "zb": zb.astype(np.float32),
        "dd": dd.astype(np.float32), "wo": wo.astype(np.float32),
        "maskn": masks, "ident": ident, "sel": sel,
    }


def _mask_ident():
    masks = np.zeros((128, 2, Q), np.float32)
    for st in range(2):
        s_idx = st * 128 + np.arange(128)[:, None]
        l_idx = np.arange(Q)[None, :]
        masks[:, st, :][s_idx > l_idx] = NEG
    ident = np.eye(128, dtype=np.float32)
    return masks, ident


def kernel(u, W_in, conv_w, conv_b, z_bias, D, W_out, _want_results=False, _trace=False, _debug=False):
    u = np.asarray(u, np.float32)
    W_in = np.asarray(W_in, np.float32)
    conv_w = np.asarray(conv_w, np.float32)
    conv_b = np.asarray(conv_b, np.float32)
    z_bias = np.asarray(z_bias, np.float32)
    D = np.asarray(D, np.float32)
    W_out = np.asarray(W_out, np.float32)

    nc = _build_program(debug=True) if _debug else _get_program()

    masks, ident = _mask_ident()
    sel = np.zeros((8, NHC, 128), np.float32)
    for h in range(NHC):
        sel[h, h, :] = 1.0

    in_maps = [_pack_core_inputs(c, u, W_in, conv_w, conv_b, z_bias, D, W_out, masks, ident, sel)
               for c in range(8)]
    kw = {}
    if _trace:
        kw = {"trace": True}
    res = run_bass_kernel_spmd(nc, in_maps, core_ids=list(range(8)), **kw)

    out = np.empty((B_SZ, L, D_MODEL), np.float32)
    for c in range(8):
        b, g = c // 4, c % 4
        out[b, :, g * 512:(g + 1) * 512] = res.results[c]["out"]
    if _want_results:
        return out, res
    return out

